# revision 16
# baseline (speedup 1.0000x reference)
"""Trainium2 Bass kernel for a 2-layer GCN with data-aware attention gate.

Math (per reference):
    src,dst = edges + self-loops; deg = bincount(dst); dinv = rsqrt(deg)
    norm = dinv[src]*dinv[dst]
    h1 = relu(segsum(norm * (x@W1)[src], dst) + b1)
    h2 = relu(segsum(norm * (h1@W2)[src], dst) + b2)
    out = h2 * sigmoid(h2@attn_w + attn_b)

Device strategy (8 NeuronCores, node/dst-sharded):
    norm factorizes: agg[d] = dinv[d] * sum_{e->d} (dinv[s] * T[s]).
    Per layer: each core computes T' = dinv .* (H @ W) for its node shard
    (x is staged transposed so the transform is a single matmul; the dinv
    row-scale commutes through the right-multiplication and is applied on
    the PSUM->SBUF copy), AllGather of bf16 T' shards, then per-edge
    dma_gather of row PAIRS (256B) from the [TOT/2, 128]-strided view of
    the [TOT, 64] bf16 table and PE one-hot selection-matrix matmuls
    accumulate 128-slot window segment sums in PSUM.  The one-hot S
    matrices are PRECOMPUTED ON HOST and streamed from DRAM by bulk
    contiguous DMA on the otherwise-idle Scalar engine's HWDGE ring, so
    no compute engine builds them.  Layer-1 table rows are plain bf16 T1'
    (64 vals = 128B); layer-2 rows are hi/lo bf16 of T2' (2x32 vals =
    128B), so both layers share the same gather geometry.  int16 gather
    indices address pairs (s_pos>>1 <= 25088), with edges split by
    source-position parity choosing the 64-col half of each gathered
    pair.  Host-side prep deals nodes into windows (LPT on degree) and
    pads per-(window,parity) edge counts to the same 128-multiple across
    cores so the single SPMD instruction stream is valid for all 8 cores.
"""

import sys

import numpy as np

_CONC = "/opt/trn_rl_repo"
if _CONC not in sys.path:
    sys.path.insert(0, _CONC)

# ---------------------------------------------------------------------------
# configuration
# ---------------------------------------------------------------------------


class Cfg:
    def __init__(self, N=50000, DIN=128, DH=64, DOUT=32, NC=8, WPC=50, WPG=5):
        self.N, self.DIN, self.DH, self.DOUT = N, DIN, DH, DOUT
        self.NC, self.WPC, self.WPG = NC, WPC, WPG
        assert WPC % WPG == 0
        self.G = WPC // WPG            # gather groups per core
        self.NPC = WPC * 128           # slots per core
        self.TOT = NC * self.NPC       # total slots
        assert self.TOT // 2 <= 32768
        assert DH * 2 * 2 == 256       # gather elem (row pair) must be 256B
        assert self.N <= self.TOT - 2


FULL = Cfg()

# ---------------------------------------------------------------------------
# host-side graph prep
# ---------------------------------------------------------------------------


def _assign_slots(deg, cfg):
    """LPT-deal nodes into NC*WPC bins of <=128 slots, balancing edge load.
    Returns pos[node] -> global slot position."""
    import heapq

    nbins = cfg.NC * cfg.WPC
    cap = np.full(nbins, 128, np.int64)
    order = np.argsort(-deg, kind="stable")
    heap = [(0, b) for b in range(nbins)]
    heapq.heapify(heap)
    count = np.zeros(nbins, np.int64)
    pos = np.empty(cfg.N, np.int64)
    for n in order:
        load, b = heapq.heappop(heap)
        pos[n] = b * 128 + count[b]
        count[b] += 1
        if count[b] < cap[b]:
            heapq.heappush(heap, (load + int(deg[n]), b))
    return pos


def prep(x, edge_index, cfg):
    """Build per-core input arrays and the static (SPMD-uniform) chunk plan."""
    import ml_dtypes
    N, NC, WPC, WPG, G = cfg.N, cfg.NC, cfg.WPC, cfg.WPG, cfg.G
    NPC, DIN = cfg.NPC, cfg.DIN
    bf16 = ml_dtypes.bfloat16

    loops = np.arange(N, dtype=np.int64)
    src = np.concatenate([edge_index[0].astype(np.int64), loops])
    dst = np.concatenate([edge_index[1].astype(np.int64), loops])
    deg = np.bincount(dst, minlength=N).astype(np.float32)
    dinv = (1.0 / np.sqrt(np.maximum(deg, 1e-12))).astype(np.float32)

    pos = _assign_slots(deg, cfg)

    # per-core transposed x shard + per-slot dinv
    x_shT = np.zeros((NC, DIN, NPC), np.float32)
    dinv_slot = np.ones((NC, 128, WPC), np.float32)
    node_of = np.full(cfg.TOT, -1, np.int64)
    node_of[pos] = np.arange(N)
    xf = np.asarray(x, np.float32)
    for c in range(NC):
        seg = node_of[c * NPC:(c + 1) * NPC]
        m = seg >= 0
        blk = np.zeros((NPC, DIN), np.float32)
        blk[m] = xf[seg[m]]
        x_shT[c] = np.ascontiguousarray(blk.T)
        dv = np.ones(NPC, np.float32)
        dv[m] = dinv[seg[m]]
        dinv_slot[c] = dv.reshape(WPC, 128).T

    # edge records
    s_pos = pos[src]
    d_pos = pos[dst]
    c_e = d_pos // NPC
    w_e = (d_pos % NPC) // 128          # window within core
    slot_e = (d_pos % 128).astype(np.int64)
    half_e = (s_pos & 1).astype(np.int64)
    gidx_e = (s_pos >> 1).astype(np.int64)

    # bucket edges by (core, window, half)
    buckets = {}
    key_all = (c_e * WPC + w_e) * 2 + half_e
    order_e = np.argsort(key_all, kind="stable")
    ks = key_all[order_e]
    bounds = np.searchsorted(ks, np.arange(NC * WPC * 2 + 1))
    for key in range(NC * WPC * 2):
        lo, hi = bounds[key], bounds[key + 1]
        if hi > lo:
            buckets[key] = order_e[lo:hi]

    # per-(window,half) 128-aligned target, equalized across cores
    tgt = np.zeros((WPC, 2), np.int64)
    for w in range(WPC):
        for h in range(2):
            mx = max(len(buckets.get((c * WPC + w) * 2 + h, ()))
                     for c in range(NC))
            tgt[w, h] = int(np.ceil(max(mx, 1) / 128) * 128)

    # per-(group,half) gather segment = concat of member windows' segments
    seglen = np.zeros((G, 2), np.int64)
    for g in range(G):
        for h in range(2):
            seglen[g, h] = tgt[g * WPG:(g + 1) * WPG, h].sum()

    idx_cols = int(sum(seglen[g, h] // 16 for g in range(G) for h in range(2)))
    chunk_tot = int(sum(seglen[g, h] // 128 for g in range(G) for h in range(2)))
    idx_all = np.zeros((NC, 128, idx_cols), np.int16)
    s_all = np.zeros((NC, 128, chunk_tot * 128), bf16)

    ioff, coff = {}, {}
    io = co = 0
    for g in range(G):
        for h in range(2):
            ioff[(g, h)] = io
            coff[(g, h)] = co
            io += int(seglen[g, h]) // 16
            co += int(seglen[g, h]) // 128
    # chunk column (within s_all / gather tile) of window w's half-h run
    wcol = np.zeros((WPC, 2), np.int64)
    for g in range(G):
        for h in range(2):
            c0 = coff[(g, h)]
            for wl in range(WPG):
                w = g * WPG + wl
                wcol[w, h] = c0
                c0 += tgt[w, h] // 128

    ones = np.ones(1, bf16)[0]
    for c in range(NC):
        for g in range(G):
            for h in range(2):
                n = int(seglen[g, h])
                gi = np.zeros(n, np.int64)
                sl = np.full(n, -1, np.int64)
                p = 0
                for wl in range(WPG):
                    w = g * WPG + wl
                    es = buckets.get((c * WPC + w) * 2 + h, ())
                    if len(es):
                        es = es[np.argsort(gidx_e[es], kind="stable")]
                    ne = len(es)
                    gi[p:p + ne] = gidx_e[es]
                    sl[p:p + ne] = slot_e[es]
                    p += int(tgt[w, h])
                wrapped = gi.reshape(n // 16, 16).T.astype(np.int16)
                idx_all[c, :, ioff[(g, h)]:ioff[(g, h)] + n // 16] = np.tile(
                    wrapped, (8, 1))
                # one-hot S: edge at (chunk k, partition p) -> column slot
                ei = np.arange(n)[sl >= 0]
                s_all[c, ei % 128,
                      (coff[(g, h)] + ei // 128) * 128 + sl[sl >= 0]] = ones

    plan = dict(tgt=tgt, seglen=seglen, ioff=ioff, coff=coff, wcol=wcol,
                idx_cols=idx_cols, chunk_tot=chunk_tot)
    host = dict(x_shT=x_shT, dinv_slot=dinv_slot, idx_all=idx_all,
                s_all=s_all, pos=pos)
    return plan, host


# ---------------------------------------------------------------------------
# device kernel
# ---------------------------------------------------------------------------


def build(cfg, plan):
    import concourse.bass as bass
    import concourse.mybir as mybir
    import concourse.tile as tile
    from concourse import bacc

    NC, WPC, WPG, G = cfg.NC, cfg.WPC, cfg.WPG, cfg.G
    NPC, TOT, DIN, DH, DOUT = cfg.NPC, cfg.TOT, cfg.DIN, cfg.DH, cfg.DOUT
    f32 = mybir.dt.float32
    bf16 = mybir.dt.bfloat16
    tgt, seglen = plan["tgt"], plan["seglen"]
    ioff, coff, wcol = plan["ioff"], plan["coff"], plan["wcol"]
    AF = mybir.ActivationFunctionType
    PW = 2 * DH                     # gathered pair width in bf16 elems
    MULT = mybir.AluOpType.mult
    ADD = mybir.AluOpType.add
    SUB = mybir.AluOpType.subtract

    nc = bacc.Bacc(
        "TRN2", target_bir_lowering=False, debug=False,
        num_devices=NC, num_swdge_queues=4,
    )

    # I/O
    xT_d = nc.dram_tensor("xT_sh", [DIN, NPC], f32, kind="ExternalInput")
    w1_d = nc.dram_tensor("w1", [DIN, DH], f32, kind="ExternalInput")
    w2_d = nc.dram_tensor("w2", [DH, DOUT], f32, kind="ExternalInput")
    b1_d = nc.dram_tensor("b1rep", [128, DH], f32, kind="ExternalInput")
    b2_d = nc.dram_tensor("b2rep", [128, DOUT], f32, kind="ExternalInput")
    aw_d = nc.dram_tensor("awrep", [128, DOUT], f32, kind="ExternalInput")
    ab_d = nc.dram_tensor("abcol", [128, 1], f32, kind="ExternalInput")
    dv_d = nc.dram_tensor("dinv_slot", [128, WPC], f32, kind="ExternalInput")
    dw64_d = nc.dram_tensor("dinv_w64", [128, WPC * DH], f32,
                            kind="ExternalInput")
    dw32_d = nc.dram_tensor("dinv_w32", [128, WPC * DOUT], f32,
                            kind="ExternalInput")
    b1w_d = nc.dram_tensor("b1w", [128, WPG * DH], f32, kind="ExternalInput")
    b2w_d = nc.dram_tensor("b2w", [128, WPG * DOUT], f32,
                           kind="ExternalInput")
    aww_d = nc.dram_tensor("aww", [128, WPG * DOUT], f32,
                           kind="ExternalInput")
    id_d = nc.dram_tensor("ident", [128, 128], f32, kind="ExternalInput")
    ix_d = nc.dram_tensor("idx_all", [128, plan["idx_cols"]], mybir.dt.int16,
                          kind="ExternalInput")
    s_d = nc.dram_tensor("s_all", [128, plan["chunk_tot"] * 128], bf16,
                         kind="ExternalInput")
    out_d = nc.dram_tensor("out_sh", [NPC, DOUT], f32, kind="ExternalOutput")

    rg = [list(range(NC))]

    with tile.TileContext(nc) as tc:
        with tc.tile_pool(name="const", bufs=1) as cpool:
            def load(dram, shape, dt=f32):
                t = cpool.tile(shape, dt, tag=dram.name, name=dram.name + "_s")
                nc.sync.dma_start(t[:], dram.ap())
                return t

            w1_s = load(w1_d, [DIN, DH])
            w2_s = load(w2_d, [DH, DOUT])
            b1_s = load(b1_d, [128, DH])
            b2_s = load(b2_d, [128, DOUT])
            aw_s = load(aw_d, [128, DOUT])
            ab_s = load(ab_d, [128, 1])
            dv_s = load(dv_d, [128, WPC])
            dw64_s = load(dw64_d, [128, WPC * DH])
            dw32_s = load(dw32_d, [128, WPC * DOUT])
            b1w_s = load(b1w_d, [128, WPG * DH])
            b2w_s = load(b2w_d, [128, WPG * DOUT])
            aww_s = load(aww_d, [128, WPG * DOUT])
            id_s = load(id_d, [128, 128])
            ix_s = load(ix_d, [128, plan["idx_cols"]], mybir.dt.int16)

            with tc.tile_pool(name="dram", bufs=1, space="DRAM") as dpool:
                t1_shard = dpool.tile([NPC, DH], bf16, tag="t1s", name="t1s")
                t1_full = dpool.tile([TOT, DH], bf16, tag="t1f",
                                     name="t1f", addr_space="Shared")
                t2_shard = dpool.tile([NPC, DH], bf16, tag="t2s", name="t2s")
                t2_full = dpool.tile([TOT, DH], bf16, tag="t2f",
                                     name="t2f", addr_space="Shared")

                # ---- phase 1: T1' = dinv .* (x @ W1), node-major
                # (x staged transposed; dinv row-scale commutes through @W1)
                with (
                    tc.tile_pool(name="tf_in", bufs=6) as pin,
                    tc.tile_pool(name="tf_ps", bufs=2, space="PSUM") as pps,
                    tc.tile_pool(name="tf_sb", bufs=3) as psb,
                ):
                    for g in range(G):
                        hp = pps.tile([128, WPG * DH], f32, tag="hp",
                                      name="hp")
                        for wl in range(WPG):
                            w = g * WPG + wl
                            xt = pin.tile([128, 128], f32, tag="xt",
                                          name="xt")
                            nc.sync.dma_start(
                                xt[:], xT_d.ap()[:, w * 128:(w + 1) * 128])
                            nc.tensor.matmul(
                                hp[:, wl * DH:(wl + 1) * DH], lhsT=xt[:],
                                rhs=w1_s[:], start=True, stop=True)
                        t1b = psb.tile([128, WPG * DH], bf16, tag="t1b",
                                       name="t1b")
                        nc.vector.tensor_mul(
                            out=t1b[:], in0=hp[:],
                            in1=dw64_s[:, g * WPG * DH:(g + 1) * WPG * DH])
                        for wl in range(WPG):
                            w = g * WPG + wl
                            nc.sync.dma_start(
                                t1_shard[w * 128:(w + 1) * 128, :],
                                t1b[:, wl * DH:(wl + 1) * DH])

                # ---- phase 2: AllGather layer-1 table
                nc.gpsimd.collective_compute(
                    "AllGather", mybir.AluOpType.bypass, replica_groups=rg,
                    ins=[t1_shard[:]], outs=[t1_full[:]],
                )

                # ---- aggregation: gather row pairs + one-hot matmul segsums
                def aggregate(full, flush_fn, qctr=[0]):
                    fv = full.rearrange("(a b) d -> a (b d)", b=2)
                    with (
                        tc.tile_pool(name="gpool", bufs=3) as gp,
                        tc.tile_pool(name="spool", bufs=2) as sp,
                        tc.tile_pool(name="apsum", bufs=4, space="PSUM") as aps,
                    ):
                        for g in range(G):
                            gts = {}
                            sts = {}
                            for h in range(2):
                                n = int(seglen[g, h])
                                nch = n // 128
                                # host-precomputed one-hot S for the whole
                                # (group, half), streamed on the Scalar DGE
                                co = coff[(g, h)]
                                St = sp.tile([128, nch * 128], bf16,
                                             tag=f"S{h}", name=f"S{h}")
                                nc.scalar.dma_start(
                                    St[:], s_d.ap()[:, co * 128:
                                                    (co + nch) * 128])
                                sts[h] = St
                                gt = gp.tile([128, nch * PW], bf16,
                                             tag=f"g{h}", name=f"gt{h}")
                                io = ioff[(g, h)]
                                n1 = (n // 256) * 128
                                for (o0, nn) in ((0, n1), (n1, n - n1)):
                                    if nn == 0:
                                        continue
                                    nc.gpsimd.dma_gather(
                                        out_ap=gt[:, o0 * PW // 128:
                                                  (o0 + nn) * PW // 128]
                                        .rearrange("p (c d) -> p c d", d=PW),
                                        in_ap=fv[:, :],
                                        idxs_ap=ix_s[:, io + o0 // 16:
                                                     io + (o0 + nn) // 16],
                                        num_idxs=nn, num_idxs_reg=nn,
                                        elem_size=PW, elem_step=PW,
                                        queue_num=qctr[0] % 4,
                                        single_packet=False,
                                    )
                                    qctr[0] += 1
                                gts[h] = gt
                            ps = aps.tile([128, WPG * DH], f32, tag="agg",
                                          name="agg")
                            for wl in range(WPG):
                                w = g * WPG + wl
                                nmm = (int(tgt[w, 0]) + int(tgt[w, 1])) // 128
                                j = 0
                                for h in range(2):
                                    nch = int(tgt[w, h]) // 128
                                    tcol = int(wcol[w, h] - coff[(g, h)])
                                    for k in range(nch):
                                        base = (tcol + k) * PW + h * DH
                                        nc.tensor.matmul(
                                            ps[:, wl * DH:(wl + 1) * DH],
                                            lhsT=sts[h][:, (tcol + k) * 128:
                                                        (tcol + k + 1) * 128],
                                            rhs=gts[h][:, base:base + DH],
                                            start=(j == 0),
                                            stop=(j == nmm - 1),
                                        )
                                        j += 1
                            flush_fn(g, ps)

                # ---- layer-1 flush: h=relu(dinv*agg+b1); T2'=dinv.*(h@W2)
                with (
                    tc.tile_pool(name="fl_sb", bufs=3) as fsb,
                    tc.tile_pool(name="fl_ps", bufs=2, space="PSUM") as fps,
                ):
                    def flush1(g, ps):
                        GW = WPG * DH
                        v2 = fsb.tile([128, GW], f32, tag="v2", name="v2")
                        nc.vector.tensor_mul(
                            out=v2[:], in0=ps[:],
                            in1=dw64_s[:, g * GW:(g + 1) * GW])
                        v2b = fsb.tile([128, GW], f32, tag="v2b", name="v2b")
                        nc.vector.tensor_add(
                            out=v2b[:], in0=v2[:], in1=b1w_s[:])
                        h2 = fsb.tile([128, GW], f32, tag="h2", name="h2")
                        nc.scalar.activation(h2[:], v2b[:], func=AF.Relu)
                        hts = fsb.tile([DH, WPG * 128], f32, tag="hts",
                                       name="hts")
                        for wl in range(WPG):
                            htp = fps.tile([DH, 128], f32, tag="htp",
                                           name="htp")
                            nc.tensor.transpose(
                                htp[:], h2[:, wl * DH:(wl + 1) * DH],
                                id_s[:])
                            nc.vector.tensor_copy(
                                hts[:, wl * 128:(wl + 1) * 128], htp[:])
                        GO = WPG * DOUT
                        t2p = fps.tile([128, GO], f32, tag="t2p", name="t2p")
                        for wl in range(WPG):
                            nc.tensor.matmul(
                                t2p[:, wl * DOUT:(wl + 1) * DOUT],
                                lhsT=hts[:, wl * 128:(wl + 1) * 128],
                                rhs=w2_s[:], start=True, stop=True)
                        # t2 rows = [hi(32) | lo(32)] bf16 of dinv.*t2p
                        m_all = fsb.tile([128, GO], f32, tag="m_all",
                                         name="m_all")
                        nc.vector.tensor_mul(
                            out=m_all[:], in0=t2p[:],
                            in1=dw32_s[:, g * GO:(g + 1) * GO])
                        t2g = fsb.tile([128, WPG * DH], bf16, tag="t2g",
                                       name="t2g")
                        t2v = t2g[:].rearrange("p (w d) -> p w d", d=DH)
                        nc.vector.tensor_copy(
                            t2v[:, :, :DOUT],
                            m_all[:].rearrange("p (w d) -> p w d", d=DOUT))
                        hib = fsb.tile([128, GO], f32, tag="hib", name="hib")
                        nc.vector.tensor_copy(
                            hib[:].rearrange("p (w d) -> p w d", d=DOUT),
                            t2v[:, :, :DOUT])
                        nc.vector.tensor_sub(
                            out=t2v[:, :, DOUT:], in0=m_all[:].rearrange(
                                "p (w d) -> p w d", d=DOUT),
                            in1=hib[:].rearrange("p (w d) -> p w d", d=DOUT))
                        for wl in range(WPG):
                            w = g * WPG + wl
                            nc.sync.dma_start(
                                t2_shard[w * 128:(w + 1) * 128, :],
                                t2g[:, wl * DH:(wl + 1) * DH])

                    aggregate(t1_full[:], flush1)

                    # ---- phase 4: AllGather layer-2 table
                    nc.gpsimd.collective_compute(
                        "AllGather", mybir.AluOpType.bypass, replica_groups=rg,
                        ins=[t2_shard[:]], outs=[t2_full[:]],
                    )

                    # ---- layer-2 flush: h2 + attention gate -> out
                    def flush2(g, ps):
                        GO = WPG * DOUT
                        psv = ps[:].rearrange("p (w d) -> p w d", d=DH)
                        lo2 = fsb.tile([128, GO], f32, tag="f2lo",
                                       name="f2lo")
                        nc.vector.tensor_copy(
                            lo2[:].rearrange("p (w d) -> p w d", d=DOUT),
                            psv[:, :, DOUT:])
                        agg = fsb.tile([128, GO], f32, tag="f2agg",
                                       name="f2agg")
                        nc.vector.tensor_add(
                            out=agg[:].rearrange("p (w d) -> p w d", d=DOUT),
                            in0=psv[:, :, :DOUT],
                            in1=lo2[:].rearrange("p (w d) -> p w d", d=DOUT))
                        v2 = fsb.tile([128, GO], f32, tag="f2v2",
                                      name="f2v2")
                        nc.vector.tensor_mul(
                            out=v2[:], in0=agg[:],
                            in1=dw32_s[:, g * GO:(g + 1) * GO])
                        v2b = fsb.tile([128, GO], f32, tag="f2v2b",
                                       name="f2v2b")
                        nc.vector.tensor_add(
                            out=v2b[:], in0=v2[:], in1=b2w_s[:])
                        hh = fsb.tile([128, GO], f32, tag="f2h", name="f2h")
                        nc.scalar.activation(hh[:], v2b[:], func=AF.Relu)
                        a = fsb.tile([128, GO], f32, tag="f2a", name="f2a")
                        nc.vector.tensor_mul(out=a[:], in0=hh[:],
                                             in1=aww_s[:])
                        ar = fsb.tile([128, WPG], f32, tag="f2ar",
                                      name="f2ar")
                        nc.vector.tensor_reduce(
                            ar[:].rearrange("p (w o) -> p w o", o=1),
                            a[:].rearrange("p (w d) -> p w d", d=DOUT),
                            axis=mybir.AxisListType.X,
                            op=mybir.AluOpType.add)
                        at = fsb.tile([128, WPG], f32, tag="f2at",
                                      name="f2at")
                        nc.scalar.activation(at[:], ar[:], func=AF.Sigmoid,
                                             bias=ab_s[:, :1])
                        for wl in range(WPG):
                            w = g * WPG + wl
                            o = fsb.tile([128, DOUT], f32, tag="f2o",
                                         name="f2o")
                            nc.vector.tensor_scalar_mul(
                                o[:], hh[:, wl * DOUT:(wl + 1) * DOUT],
                                at[:, wl:wl + 1])
                            nc.sync.dma_start(
                                out_d.ap()[w * 128:(w + 1) * 128, :], o[:])

                    aggregate(t2_full[:], flush2)

    nc.compile()
    return nc


# ---------------------------------------------------------------------------
# entry point
# ---------------------------------------------------------------------------


def _make_in_maps(cfg, host, W1, b1, W2, b2, attn_w, attn_b):
    NC = cfg.NC
    ident = np.eye(128, dtype=np.float32)
    in_maps = []
    for c in range(NC):
        in_maps.append({
            "xT_sh": host["x_shT"][c],
            "w1": np.asarray(W1, np.float32),
            "w2": np.asarray(W2, np.float32),
            "b1rep": np.tile(np.asarray(b1, np.float32), (128, 1)),
            "b2rep": np.tile(np.asarray(b2, np.float32), (128, 1)),
            "awrep": np.tile(np.asarray(attn_w, np.float32).reshape(1, -1),
                             (128, 1)),
            "abcol": np.full((128, 1),
                             np.asarray(attn_b, np.float32).reshape(-1)[0],
                             np.float32),
            "dinv_slot": host["dinv_slot"][c],
            "dinv_w64": np.repeat(host["dinv_slot"][c], 64, axis=1),
            "dinv_w32": np.repeat(host["dinv_slot"][c], 32, axis=1),
            "b1w": np.tile(np.asarray(b1, np.float32), (128, cfg.WPG)),
            "b2w": np.tile(np.asarray(b2, np.float32), (128, cfg.WPG)),
            "aww": np.tile(np.asarray(attn_w, np.float32).reshape(1, -1),
                           (128, cfg.WPG)),
            "ident": ident,
            "idx_all": host["idx_all"][c],
            "s_all": host["s_all"][c],
        })
    return in_maps


def run(x, edge_index, W1, b1, W2, b2, attn_w, attn_b, cfg=None,
        backend="hw", trace=False):
    cfg = cfg or FULL
    plan, host = prep(x, edge_index, cfg)
    nc = build(cfg, plan)
    in_maps = _make_in_maps(cfg, host, W1, b1, W2, b2, attn_w, attn_b)

    if backend == "sim":
        from concourse.bass_interp import MultiCoreSim
        sim = MultiCoreSim(nc, num_cores=cfg.NC, trace=False)
        for c, core in enumerate(sim.cores.values()):
            for name, arr in in_maps[c].items():
                core.tensor(name)[:] = arr
        sim.simulate()
        outs = [core.tensor("out_sh").copy() for core in sim.cores.values()]
        exec_ns = None
    else:
        from concourse import bass_utils
        from concourse.bass_interp import get_hw_module
        old = nc.m
        nc.m = get_hw_module(nc.m)
        try:
            res = bass_utils.run_bass_kernel_spmd(
                nc, in_maps, core_ids=list(range(cfg.NC)), trace=trace)
        finally:
            nc.m = old
        outs = [res.results[c]["out_sh"] for c in range(cfg.NC)]
        exec_ns = res.exec_time_ns

    full = np.concatenate(outs, axis=0)  # [TOT, DOUT] in slot order
    out = full[host["pos"]]              # unpermute -> [N, DOUT]
    return np.ascontiguousarray(out), exec_ns


def kernel(x, edge_index, W1, b1, W2, b2, attn_w, attn_b):
    out, _ = run(x, edge_index, W1, b1, W2, b2, attn_w, attn_b,
                 cfg=FULL, backend="hw", trace=False)
    return out
